# revision 53
# baseline (speedup 1.0000x reference)
"""Trainium2 Bass kernel for nn_BiLSTM_7928509628689.

Masked bidirectional LSTM over N=2048 ragged sequences (T<=64, D=512, H=256),
returning concat of final fwd/bwd hidden states [N, 2H].

Strategy (8 NeuronCores, data-parallel over N, 256 seqs/core):
  * Sequences globally sorted by length (desc), dealt round-robin to cores,
    right-aligned in time, so at step s only the V_s longest sequences are
    active; V_s is baked into the program and is nondecreasing.
  * x tokens stored COLUMN-PACKED across steps: dram [128, 5, sum(V_s)],
    chunk k<4 = input dims (dd = 4*p + k), chunk 4 = bias/mask rows
    (row0 = ones, row1 = pad flag -> adds b and FORCE to the gates).
  * Steps with V_s < 82 (under the ~34ns/matmul issue floor) are packed
    into blocks of <= 128 columns: per block, one x-projection pass of 8
    regions x 5 chunks covers all its steps; the recurrent projection then
    accumulates per-step into column subranges. Blocks ping-pong between
    two 2-bank PSUM sets per dir, and the x-projection of block k+2 is
    emitted right after block k's steps so the in-order PE always has
    fill work during the per-step latency chains.
  * Steps with V_s >= 82 run per-step in a 4-bank-per-dir layout (region
    stride 256). Matmul order is x(g-bank first), then h; the next step's
    x matmuls overlap this step's ACT reads bank-by-bank.
  * PSUM region order (i,i,f,f,o,o,g,g): one 6-range sigmoid covers i,f,o
    and one 2-range tanh covers g. ACT/DVE ops are per-dir with fwd-first
    priority ordering (tg_f, sig_f, tg_b, tc_f, sig_b, tc_b) so the fwd
    recurrent matmuls of the next step unblock as early as possible.
  * All state transposed: h [128, 2*KH*NS] bf16, c [...] f32, in place.
    Final step writes f32 outputs, DMA'd out per dir.

kernel(**inputs) takes the FULL unsharded inputs and returns [2048, 512] f32.
"""
import numpy as np

import concourse.tile as tile
from concourse import bacc, mybir
from concourse.bass_utils import run_bass_kernel_spmd
import bass_rust

F32 = mybir.dt.float32
BF16 = mybir.dt.bfloat16
FP8 = mybir.dt.float8e4
DR = mybir.MatmulPerfMode.DoubleRow
AF = mybir.ActivationFunctionType
OP = mybir.AluOpType

USE_DR = True              # fp8 DoubleRow recurrent projection for V >= DRTHR
DRTHR = 160

N, T, D, H = 2048, 64, 512, 256
NCORES = 8
NS = N // NCORES           # 256 sequences per core
FH = 4 * H                 # 1024 gate rows
KD = D // 128              # 4 x-projection K chunks
KX = KD + 1                # + bias/mask chunk
KH = H // 128              # 2 h-projection K chunks
FORCE = -40.0              # gate penalty for pad steps
DIRS = ("f", "b")
# region r holds gate rows ROWMAP[r]:+128 : regions (i,i,f,f,o,o,g,g)
ROWMAP = [0, 128, 256, 384, 768, 896, 512, 640]
BLKW = 128                 # block column budget (2 PSUM banks per dir)
VTHR = 82                  # pack steps with V < VTHR (matmul issue floor)

_NC_CACHE = {}


def _inst(r):
    return getattr(r, "ins", r)


def _plan(t_steps, V):
    """Partition steps into packed blocks (V < VTHR) and per-step era."""
    blocks = []   # [s0, [v...], [intra off...], W]
    rsteps = []
    cur = None
    for s in range(t_steps):
        v = int(V[s])
        if v < VTHR:
            if cur is None or cur[3] + v > BLKW:
                cur = [s, [], [], 0]
                blocks.append(cur)
            cur[1].append(v)
            cur[2].append(cur[3])
            cur[3] += v
        else:
            cur = None
            rsteps.append(s)
    # single steps with V <= 128 fit 8 regions in 2 banks: run them on a
    # per-step parity ping-pong so next-step x matmuls never WAR-wait on
    # this step's ACT reads (the conflict lags a full step instead)
    keep = []
    psteps = []
    for b in blocks:
        if len(b[1]) >= 2:
            keep.append(b)
        else:
            psteps.append(b[0])
    rsteps += psteps
    rsteps.sort()
    return keep, [], rsteps


def _build(t_steps, V):
    import contextlib

    nc = bacc.Bacc("TRN2", target_bir_lowering=False, debug=False)
    V = [int(v) for v in V]
    OFF = np.concatenate([[0], np.cumsum(V)]).astype(int)
    TOT = int(OFF[t_steps])
    blocks, psteps, rsteps = _plan(t_steps, V)

    use_dr = USE_DR and any(V[s] >= DRTHR for s in range(t_steps))

    xp_d, wih_d, whh_d, whh8_d, out_d = {}, {}, {}, {}, {}
    for d in DIRS:
        xp_d[d] = nc.dram_tensor(
            f"xp{d}", [128, KX, TOT], BF16, kind="ExternalInput"
        ).ap()
        wih_d[d] = nc.dram_tensor(
            f"wih{d}", [128, KX, FH], BF16, kind="ExternalInput"
        ).ap()
        whh_d[d] = nc.dram_tensor(
            f"whh{d}", [128, KH, FH], BF16, kind="ExternalInput"
        ).ap()
        if use_dr:
            whh8_d[d] = nc.dram_tensor(
                f"whh8{d}", [128, KH, FH], FP8, kind="ExternalInput"
            ).ap()
        out_d[d] = nc.dram_tensor(
            f"hT{d}", [128, KH * NS], F32, kind="ExternalOutput"
        ).ap()

    with tile.TileContext(nc) as tc:
        with contextlib.ExitStack() as ctx:
            wpool = ctx.enter_context(tc.tile_pool(name="w", bufs=1))
            xpool = ctx.enter_context(tc.tile_pool(name="x", bufs=3))
            xbpool = ctx.enter_context(tc.tile_pool(name="xb", bufs=1))
            spool = ctx.enter_context(tc.tile_pool(name="state", bufs=1))
            opool = ctx.enter_context(tc.tile_pool(name="outs", bufs=1))
            apool = ctx.enter_context(tc.tile_pool(name="acts", bufs=2))
            pspool = ctx.enter_context(tc.tile_pool(name="ps", bufs=1, space="PSUM"))

            # warm-up weights first so the PE p-state ramp starts ASAP
            wrm = wpool.tile([128, 512], BF16, tag="warm", name="warm")
            nc.gpsimd.memset(wrm[:], 0.0)

            wih_t, whh_t, whh8_t = {}, {}, {}
            for d in DIRS:
                wih_t[d] = wpool.tile([128, KX, FH], BF16, tag=f"wih_{d}", name=f"wih_{d}")
                whh_t[d] = wpool.tile([128, KH, FH], BF16, tag=f"whh_{d}", name=f"whh_{d}")
                if use_dr:
                    whh8_t[d] = wpool.tile(
                        [128, KH, FH], FP8, tag=f"whh8_{d}", name=f"whh8_{d}"
                    )

            # startup inputs split by direction across the two DMA queues:
            # gpsimd feeds dir f (no SP preamble delay), sync feeds dir b,
            # so dir f's first matmuls start while dir b still streams
            blocks_tot = 0
            if blocks:
                bs_end, bvs, bofs, bW = blocks[-1]
                blocks_tot = int(OFF[bs_end] + bW)
            xall = {}
            if blocks:
                for d in DIRS:
                    xall[d] = xbpool.tile(
                        [128, KX, blocks_tot], BF16, tag=f"x_{d}", name=f"x_{d}"
                    )
            qof = {"f": nc.gpsimd, "b": nc.sync}
            for d in DIRS:
                if blocks:
                    qof[d].dma_start(xall[d][:], xp_d[d][:, :, 0:blocks_tot])
                qof[d].dma_start(wih_t[d][:], wih_d[d][:])
                qof[d].dma_start(whh_t[d][:], whh_d[d][:])

            # persistent transposed state, dir-major: [128, (d, kk, NS)]
            h_t = spool.tile([128, 2 * KH * NS], BF16, tag="h", name="h")
            nc.vector.memset(h_t[:], 0.0)
            c_t = spool.tile([128, 2 * KH * NS], F32, tag="c", name="c")
            nc.vector.memset(c_t[:], 0.0)
            h8_t = None
            if use_dr:
                h8_t = spool.tile([128, 2 * KH * NS], FP8, tag="h8", name="h8")
                nc.vector.memset(h8_t[:], 0.0)

            # single PSUM tile covering all 8 banks
            ps = pspool.tile([128, 8 * 512], F32, tag="ps", name="ps")

            # per-dir activation scratch (widths up to 256)
            tgA, sgA, tcA, t1A, t2A = {}, {}, {}, {}, {}
            for d in DIRS:
                tgA[d] = apool.tile([128, 2, 256], F32, tag=f"tg{d}", name=f"tg{d}")
                sgA[d] = apool.tile([128, 6, 256], F32, tag=f"sg{d}", name=f"sg{d}")
                tcA[d] = apool.tile([128, 2, 256], F32, tag=f"tc{d}", name=f"tc{d}")
                t1A[d] = apool.tile([128, 2, 256], F32, tag=f"t1{d}", name=f"t1{d}")
                t2A[d] = apool.tile([128, 2, 256], F32, tag=f"t2{d}", name=f"t2{d}")

            def hsl(d, kk, v):
                di = DIRS.index(d)
                base = di * KH * NS + kk * NS
                return h_t[:, base : base + v]

            def cviewd(d, v):
                di = DIRS.index(d)
                base = di * KH * NS
                return c_t[:, base : base + KH * NS].rearrange(
                    "p (k n) -> p k n", k=KH
                )[:, :, :v]

            def hviewd(d, v):
                di = DIRS.index(d)
                base = di * KH * NS
                return h_t[:, base : base + KH * NS].rearrange(
                    "p (k n) -> p k n", k=KH
                )[:, :, :v]

            def h8viewd(d, v):
                di = DIRS.index(d)
                base = di * KH * NS
                return h8_t[:, base : base + KH * NS].rearrange(
                    "p (k n) -> p k n", k=KH
                )[:, :, :v]

            rset = set(rsteps)

            def dr_step(s):
                return use_dr and s < t_steps and s in rset and V[s] >= DRTHR

            def emit_chain(s, v, psg, pss, last, split_sig=False):
                """Per-step activations+state update; psg[d]/pss[d] are the
                g / ifo PSUM views [p, r, v]. fwd-first priority ordering.
                split_sig: 3 per-bank sigmoid ops so the next step's x
                matmuls get their PSUM banks back progressively."""
                f, b = DIRS
                for d in DIRS:
                    nc.scalar.activation(tgA[d][:, :, :v], psg[d], AF.Tanh)
                    nc.scalar.activation(
                        sgA[d][:, :, :v], pss[d], AF.Sigmoid
                    )
                if last:
                    # kk-half chains so the output DMA overlaps the rest
                    for d in DIRS:
                        hfin = opool.tile(
                            [128, KH * NS], F32, tag=f"hfin{d}", name=f"hfin{d}"
                        )
                        qeng = [nc.sync, nc.scalar, nc.gpsimd, nc.sync]
                        for kk in range(KH):
                            k1 = kk + 1
                            nc.vector.tensor_tensor(
                                t1A[d][:, kk:k1, :v], sgA[d][:, kk:k1, :v],
                                tgA[d][:, kk:k1, :v], OP.mult,
                            )
                            nc.vector.tensor_tensor(
                                t2A[d][:, kk:k1, :v],
                                sgA[d][:, 2 + kk : 3 + kk, :v],
                                cviewd(d, v)[:, kk:k1, :], OP.mult,
                            )
                            nc.vector.tensor_tensor(
                                cviewd(d, v)[:, kk:k1, :],
                                t1A[d][:, kk:k1, :v], t2A[d][:, kk:k1, :v],
                                OP.add,
                            )
                            nc.scalar.activation(
                                tcA[d][:, kk:k1, :v],
                                cviewd(d, v)[:, kk:k1, :], AF.Tanh,
                            )
                            nc.vector.tensor_tensor(
                                hfin[:, kk * NS : kk * NS + v],
                                sgA[d][:, 4 + kk, :v],
                                tcA[d][:, kk, :v],
                                OP.mult,
                            )
                            for qi in range(2):
                                CH = NS // 2
                                off = kk * NS + qi * CH
                                qeng[2 * kk + qi].dma_start(
                                    out_d[d][:, off : off + CH],
                                    hfin[:, off : off + CH],
                                )
                    return
                for d in DIRS:
                    nc.vector.tensor_tensor(
                        t1A[d][:, :, :v], sgA[d][:, 0:2, :v], tgA[d][:, :, :v],
                        OP.mult,
                    )
                    nc.vector.tensor_tensor(
                        t2A[d][:, :, :v], sgA[d][:, 2:4, :v], cviewd(d, v),
                        OP.mult,
                    )
                    nc.vector.tensor_tensor(
                        cviewd(d, v), t1A[d][:, :, :v], t2A[d][:, :, :v], OP.add
                    )
                    nc.scalar.activation(tcA[d][:, :, :v], cviewd(d, v), AF.Tanh)
                    tgt = (
                        h8viewd(d, v) if dr_step(s + 1) else hviewd(d, v)
                    )
                    nc.vector.tensor_tensor(
                        tgt, sgA[d][:, 4:6, :v], tcA[d][:, :, :v],
                        OP.mult,
                    )

            # ---- PE warm-up (p-state ramp) during initial DMA window ----
            NWARM = 18
            for i in range(NWARM):
                nc.tensor.matmul(
                    ps[:, 3584:4096], wrm[:, 0:128], wrm[:],
                    start=(i == 0), stop=(i == NWARM - 1),
                )

            # ================= block era (V < VTHR) =================
            # psum: dir d, parity q: base = d*2048 + q*1024, region r at r*128
            # all blocks are contiguous packed columns: ONE big DMA per dir
            # has ~7x larger segments than per-block slices (descriptor-bound)
            BTOT = blocks_tot
            # second contiguous prefetch covering the early per-step era
            ESZ = min(2432, TOT - BTOT)
            xearly = {}
            if ESZ > 0:
                for d in DIRS:
                    xearly[d] = xbpool.tile(
                        [128, KX, ESZ], BF16, tag=f"xe_{d}", name=f"xe_{d}"
                    )
                    qof[d].dma_start(
                        xearly[d][:], xp_d[d][:, :, BTOT : BTOT + ESZ]
                    )
            if use_dr:
                for d in DIRS:
                    nc.gpsimd.dma_start(whh8_t[d][:], whh8_d[d][:])

            def dma_block(bi):
                pass

            dma_gate = [None]

            def xproj_block(bi):
                s0, _, _, W = blocks[bi]
                q = bi % 2
                for d in DIRS:
                    di = DIRS.index(d)
                    base = di * 2048 + q * 1024
                    bank_start = {}
                    for r in range(8):
                        o_ap = ps[:, base + r * 128 : base + r * 128 + W]
                        msl = slice(ROWMAP[r], ROWMAP[r] + 128)
                        for k in range(KX):
                            mm = nc.tensor.matmul(
                                o_ap, wih_t[d][:, k, msl],
                                xall[d][:, k, OFF[s0] : OFF[s0] + W],
                                start=(r % 4 == 0 and k == 0), stop=False,
                                skip_group_check=True,
                            )
                            if r % 4 == 0 and k == 0:
                                bank_start[r // 4] = _inst(mm)
                                if bi == min(2, len(blocks) - 1):
                                    dma_gate[0] = _inst(mm)
                            elif k == 0:
                                bass_rust.add_dep_helper(
                                    _inst(mm), bank_start[r // 4], sync=False,
                                    reason="psum bank group order",
                                )

            def psB(d, q, lo, hi, off, v):
                di = DIRS.index(d)
                base = di * 2048 + q * 1024
                return (
                    ps[:, base + lo * 128 : base + hi * 128]
                    .rearrange("p (r c) -> p r c", r=hi - lo)[:, :, off : off + v]
                )

            # merged-dir views/scratch for tiny block steps: fixed per-op
            # costs dominate there, so one op covers both directions
            def psBM(q, lo, hi, off, v):
                return (
                    ps[:]
                    .rearrange("p (d u) -> p d u", d=2)[
                        :, :, q * 1024 + lo * 128 : q * 1024 + hi * 128
                    ]
                    .rearrange("p d (r c) -> p d r c", r=hi - lo)[
                        :, :, :, off : off + v
                    ]
                )

            def cviewm(v):
                return c_t[:].rearrange("p (d k n) -> p d k n", d=2, k=KH)[
                    :, :, :, :v
                ]

            def hviewm(v):
                return h_t[:].rearrange("p (d k n) -> p d k n", d=2, k=KH)[
                    :, :, :, :v
                ]

            tgM = apool.tile([128, 2, 2, BLKW], F32, tag="tgM", name="tgM")
            sgM = apool.tile([128, 2, 6, BLKW], F32, tag="sgM", name="sgM")
            tcM = apool.tile([128, 2, 2, BLKW], F32, tag="tcM", name="tcM")
            t1M = apool.tile([128, 2, 2, BLKW], F32, tag="t1M", name="t1M")
            t2M = apool.tile([128, 2, 2, BLKW], F32, tag="t2M", name="t2M")

            def emit_chain_merged(q, off, v):
                nc.scalar.activation(
                    tgM[:, :, :, :v], psBM(q, 6, 8, off, v), AF.Tanh
                )
                nc.scalar.activation(
                    sgM[:, :, :, :v], psBM(q, 0, 6, off, v), AF.Sigmoid
                )
                nc.vector.tensor_tensor(
                    t1M[:, :, :, :v], sgM[:, :, 0:2, :v], tgM[:, :, :, :v],
                    OP.mult,
                )
                nc.vector.tensor_tensor(
                    t2M[:, :, :, :v], sgM[:, :, 2:4, :v], cviewm(v), OP.mult
                )
                nc.vector.tensor_tensor(
                    cviewm(v), t1M[:, :, :, :v], t2M[:, :, :, :v], OP.add
                )
                nc.scalar.activation(tcM[:, :, :, :v], cviewm(v), AF.Tanh)
                nc.vector.tensor_tensor(
                    hviewm(v), sgM[:, :, 4:6, :v], tcM[:, :, :, :v], OP.mult
                )

            if blocks:
                xproj_block(0)
                if len(blocks) > 1:
                    xproj_block(1)
            for bi, (s0, vs, offs, W) in enumerate(blocks):
                q = bi % 2
                for si, (v, off) in enumerate(zip(vs, offs)):
                    s = s0 + si
                    last = s == t_steps - 1
                    for d in DIRS:
                        di = DIRS.index(d)
                        base = di * 2048 + q * 1024
                        for r in range(8):
                            o_ap = ps[
                                :, base + r * 128 + off : base + r * 128 + off + v
                            ]
                            msl = slice(ROWMAP[r], ROWMAP[r] + 128)
                            for kk in range(KH):
                                nc.tensor.matmul(
                                    o_ap,
                                    whh_t[d][:, kk, msl],
                                    hsl(d, kk, v),
                                    start=False,
                                    stop=(kk == KH - 1),
                                    skip_group_check=True,
                                )
                    emit_chain(
                        s, v,
                        {d: psB(d, q, 6, 8, off, v) for d in DIRS},
                        {d: psB(d, q, 0, 6, off, v) for d in DIRS},
                        last,
                    )
                if bi + 2 < len(blocks):
                    xproj_block(bi + 2)


            # ========== parity-step era (V <= 128, single steps) ==========
            # block layout per step, parity alternates; xproj(s+1) is
            # emitted BEFORE hproj(s) so its PSUM WAR lags a full step
            def xproj_P(s, q, xt):
                v = V[s]
                for d in DIRS:
                    di = DIRS.index(d)
                    base = di * 2048 + q * 1024
                    bank_start = {}
                    for r in range(8):
                        o_ap = ps[:, base + r * 128 : base + r * 128 + v]
                        msl = slice(ROWMAP[r], ROWMAP[r] + 128)
                        for k in range(KX):
                            mm = nc.tensor.matmul(
                                o_ap, wih_t[d][:, k, msl], xt[d][:, k, :v],
                                start=(r % 4 == 0 and k == 0), stop=False,
                                skip_group_check=True,
                            )
                            if r % 4 == 0 and k == 0:
                                bank_start[r // 4] = _inst(mm)
                            elif k == 0:
                                bass_rust.add_dep_helper(
                                    _inst(mm), bank_start[r // 4], sync=False,
                                    reason="psum bank group order",
                                )

            def hproj_P(s, q):
                v = V[s]
                for d in DIRS:
                    di = DIRS.index(d)
                    base = di * 2048 + q * 1024
                    for r in range(8):
                        o_ap = ps[:, base + r * 128 : base + r * 128 + v]
                        msl = slice(ROWMAP[r], ROWMAP[r] + 128)
                        for kk in range(KH):
                            nc.tensor.matmul(
                                o_ap, whh_t[d][:, kk, msl], hsl(d, kk, v),
                                start=False, stop=(kk == KH - 1),
                                skip_group_check=True,
                            )

            px_tiles = {}

            def dma_pstep(s):
                px_tiles[s] = {}
                for d in DIRS:
                    t = xpool.tile(
                        [128, KX, BLKW], BF16, tag=f"xP_{d}", name=f"xP_{d}"
                    )
                    nc.sync.dma_start(
                        t[:, :, : V[s]], xp_d[d][:, :, OFF[s] : OFF[s] + V[s]]
                    )
                    px_tiles[s][d] = t

            for pi, s in enumerate(psteps):
                v = V[s]
                q = pi % 2
                if pi == 0:
                    dma_pstep(s)
                    xproj_P(s, q, px_tiles[s])
                if pi + 1 < len(psteps):
                    nx = psteps[pi + 1]
                    dma_pstep(nx)
                    xproj_P(nx, (pi + 1) % 2, px_tiles[nx])
                hproj_P(s, q)
                emit_chain(
                    s, v,
                    {d: psB(d, q, 6, 8, 0, v) for d in DIRS},
                    {d: psB(d, q, 0, 6, 0, v) for d in DIRS},
                    s == t_steps - 1,
                )
                px_tiles.pop(s, None)

            # ================= per-step era (V > 128) =================
            # psum: dir d base = d*2048, region r at r*256; banks = region pairs
            def psR(d, lo, hi, v):
                di = DIRS.index(d)
                return (
                    ps[:, di * 2048 + lo * 256 : di * 2048 + hi * 256]
                    .rearrange("p (r c) -> p r c", r=hi - lo)[:, :, :v]
                )

            def xproj_R(d, s, v, xt):
                di = DIRS.index(d)
                base = di * 2048
                starts = {}
                for bank in (3, 0, 1, 2):
                    for rr in range(2):
                        r = bank * 2 + rr
                        o_ap = ps[:, base + r * 256 : base + r * 256 + v]
                        msl = slice(ROWMAP[r], ROWMAP[r] + 128)
                        for k in range(KX):
                            mm = nc.tensor.matmul(
                                o_ap, wih_t[d][:, k, msl], xt[:, k, :v],
                                start=(rr == 0 and k == 0), stop=False,
                                skip_group_check=True,
                            )
                            if rr == 0 and k == 0:
                                starts[bank] = _inst(mm)
                            elif k == 0:
                                bass_rust.add_dep_helper(
                                    _inst(mm), starts[bank], sync=False,
                                    reason="psum bank group order",
                                )

            def hproj_R(d, v, dr):
                di = DIRS.index(d)
                base = di * 2048
                for bank in (3, 0, 1, 2):
                    for rr in range(2):
                        r = bank * 2 + rr
                        o_ap = ps[:, base + r * 256 : base + r * 256 + v]
                        msl = slice(ROWMAP[r], ROWMAP[r] + 128)
                        if dr:
                            nc.tensor.matmul(
                                o_ap, whh8_t[d][:, :, msl], h8viewd(d, v),
                                start=False, stop=(rr == 1),
                                perf_mode=DR,
                                skip_group_check=True,
                            )
                        else:
                            for kk in range(KH):
                                nc.tensor.matmul(
                                    o_ap, whh_t[d][:, kk, msl], hsl(d, kk, v),
                                    start=False,
                                    stop=(rr == 1 and kk == KH - 1),
                                    skip_group_check=True,
                                )

            for s in rsteps:
                v = V[s]
                last = s == t_steps - 1
                if OFF[s] >= BTOT and OFF[s] + v <= BTOT + ESZ:
                    xts = {d: xearly[d][:, :, OFF[s] - BTOT :] for d in DIRS}
                else:
                    xts = {}
                    for d in DIRS:
                        xts[d] = xpool.tile(
                            [128, KX, 256], BF16, tag=f"xR_{d}", name=f"xR_{d}"
                        )
                        dm = nc.sync.dma_start(
                            xts[d][:, :, :v], xp_d[d][:, :, OFF[s] : OFF[s] + v]
                        )
                        if dma_gate[0] is not None:
                            # keep bulk prefetch off the DMA hardware until
                            # the startup transfers have drained
                            bass_rust.add_dep_helper(
                                _inst(dm), dma_gate[0], sync=True,
                                reason="throttle steady-state prefetch",
                            )
                            dma_gate[0] = None
                for d in DIRS:
                    xproj_R(d, s, v, xts[d])
                for d in DIRS:
                    hproj_R(d, v, dr_step(s))
                emit_chain(
                    s, v,
                    {d: psR(d, 6, 8, v) for d in DIRS},
                    {d: psR(d, 0, 6, v) for d in DIRS},
                    last,
                )

    nc.compile()
    return nc


def _get_nc(t_steps, V):
    key = (t_steps, tuple(int(v) for v in V), USE_DR, DRTHR)
    if key not in _NC_CACHE:
        _NC_CACHE[key] = _build(t_steps, V)
    return _NC_CACHE[key]


def _prep_weights(W_ih, W_hh, b):
    """lhsT layouts for one direction, bias/mask folded as x chunk KD."""
    import ml_dtypes

    wdt = ml_dtypes.bfloat16
    wih = np.zeros((128, KX, FH), np.float32)
    wih[:, :KD, :] = W_ih.T.reshape(128, KD, FH)   # (p, k) <-> dd = KD*p + k
    coef = np.zeros(FH, np.float32)
    coef[: 2 * H] = FORCE       # i, f gates
    coef[3 * H :] = FORCE       # o gate
    wih[0, KD, :] = b.astype(np.float32)
    wih[1, KD, :] = coef
    wih = np.ascontiguousarray(wih.astype(wdt))
    whhf = W_hh.T.reshape(KH, 128, FH).transpose(1, 0, 2)
    whh = np.ascontiguousarray(whhf.astype(wdt))   # (p, kk) <-> hrow = 128*kk + p
    whh8 = np.ascontiguousarray(whhf.astype(ml_dtypes.float8_e4m3fn))
    return wih, whh, whh8


def _prep_core(seqs_c, lens_c, t_steps, V, OFF):
    """Packed per-core x arrays. seqs_c [NS, T, D], lens_c [NS] (sorted desc)."""
    import ml_dtypes

    bf16 = ml_dtypes.bfloat16
    TOTC = int(OFF[t_steps])
    shift = t_steps - lens_c                                   # pad steps/seq
    rows = np.concatenate([np.arange(V[s]) for s in range(t_steps)])
    steps = np.repeat(np.arange(t_steps), V[:t_steps])
    pad = steps < shift[rows]                                  # not yet started

    out = {}
    for dname, rev in (("f", False), ("b", True)):
        if rev:
            tok = t_steps - 1 - steps
        else:
            tok = steps - shift[rows]
        tokc = np.clip(tok, 0, T - 1)
        gat = seqs_c[rows, tokc]                               # [TOTC, D]
        if not rev:
            gat = np.where(pad[:, None], np.float32(0.0), gat)
        arr = np.zeros((128, KX, TOTC), np.float32)
        arr[:, :KD, :] = gat.T.reshape(128, KD, TOTC)          # dd = KD*p + k
        arr[0, KD, :] = 1.0
        arr[1, KD, :] = pad.astype(np.float32)
        out[dname] = np.ascontiguousarray(arr.astype(bf16))
    return out


def _unfold(hT):
    """[128, KH*NS] device tile -> [NS, H] h matrix."""
    h_rows = np.concatenate([hT[:, i * NS : (i + 1) * NS] for i in range(KH)], axis=0)
    return h_rows.T  # [NS, H]


def _run(inputs, trace=False, t_cap=None, **spmd_kwargs):
    all_embs = np.asarray(inputs["all_embs"], dtype=np.float32)
    lengths = np.asarray(inputs["lengths"]).astype(np.int64)
    starts = np.asarray(inputs["starts"]).astype(np.int64)

    if np.array_equal(starts, np.arange(N, dtype=np.int64) * T):
        seqs = all_embs.reshape(N, T, D)
    else:
        seqs = all_embs[starts[:, None] + np.arange(T)[None, :]]

    # global sort by length desc, deal round-robin to cores
    order = np.argsort(-lengths, kind="stable")
    t_steps = int(lengths.max())
    if t_cap is not None:
        t_steps = min(t_steps, t_cap)
    core_idx = [order[c::NCORES] for c in range(NCORES)]  # [NCORES][NS]

    # baked active widths: V_s = max over cores of #{len >= t_steps - s}
    Ls = np.stack([np.minimum(lengths[ci], t_steps) for ci in core_idx])  # [NC, NS]
    thr = t_steps - np.arange(t_steps)  # [t]
    V = (Ls[:, None, :] >= thr[None, :, None]).sum(-1).max(0)  # [t]
    V = np.maximum(V, 1).astype(int)
    OFF = np.concatenate([[0], np.cumsum(V)]).astype(int)

    w = {}
    for d, (wi, wh, bb) in {
        "f": (inputs["W_ih_f"], inputs["W_hh_f"], inputs["b_f"]),
        "b": (inputs["W_ih_b"], inputs["W_hh_b"], inputs["b_b"]),
    }.items():
        w[d] = _prep_weights(
            np.asarray(wi, np.float32), np.asarray(wh, np.float32),
            np.asarray(bb, np.float32),
        )

    in_maps = []
    for ci in range(NCORES):
        idx = core_idx[ci]
        m = _prep_core(seqs[idx], np.minimum(lengths[idx], t_steps), t_steps, V, OFF)
        im = {
            "xpf": m["f"], "xpb": m["b"],
            "wihf": w["f"][0], "whhf": w["f"][1],
            "wihb": w["b"][0], "whhb": w["b"][1],
        }
        if USE_DR and any(int(v) >= DRTHR for v in V):
            im["whh8f"] = w["f"][2]
            im["whh8b"] = w["b"][2]
        in_maps.append(im)

    nc = _get_nc(t_steps, V)
    res = None
    for attempt in range(3):
        try:
            res = run_bass_kernel_spmd(
                nc, in_maps, core_ids=list(range(NCORES)), trace=trace,
                **spmd_kwargs
            )
            break
        except Exception:
            # rare transient NRT_EXEC_UNIT_UNRECOVERABLE right after a
            # fresh NEFF load; a plain re-execute has always recovered
            if attempt == 2:
                raise
            import time as _time

            _time.sleep(2.0)

    out = np.empty((N, 2 * H), np.float32)
    for ci in range(NCORES):
        out[core_idx[ci], :H] = _unfold(res.results[ci]["hTf"])
        out[core_idx[ci], H:] = _unfold(res.results[ci]["hTb"])
    return out, res


def kernel(**inputs) -> np.ndarray:
    out, _ = _run(inputs)
    return out


# revision 54
# speedup vs baseline: 1.0219x; 1.0219x over previous
"""Trainium2 Bass kernel for nn_BiLSTM_7928509628689.

Masked bidirectional LSTM over N=2048 ragged sequences (T<=64, D=512, H=256),
returning concat of final fwd/bwd hidden states [N, 2H].

Strategy (8 NeuronCores, data-parallel over N, 256 seqs/core):
  * Sequences globally sorted by length (desc), dealt round-robin to cores,
    right-aligned in time, so at step s only the V_s longest sequences are
    active; V_s is baked into the program and is nondecreasing.
  * x tokens stored COLUMN-PACKED across steps: dram [128, 5, sum(V_s)],
    chunk k<4 = input dims (dd = 4*p + k), chunk 4 = bias/mask rows
    (row0 = ones, row1 = pad flag -> adds b and FORCE to the gates).
  * Steps with V_s < 82 (under the ~34ns/matmul issue floor) are packed
    into blocks of <= 128 columns: per block, one x-projection pass of 8
    regions x 5 chunks covers all its steps; the recurrent projection then
    accumulates per-step into column subranges. Blocks ping-pong between
    two 2-bank PSUM sets per dir, and the x-projection of block k+2 is
    emitted right after block k's steps so the in-order PE always has
    fill work during the per-step latency chains.
  * Steps with V_s >= 82 run per-step in a 4-bank-per-dir layout (region
    stride 256). Matmul order is x(g-bank first), then h; the next step's
    x matmuls overlap this step's ACT reads bank-by-bank.
  * PSUM region order (i,i,f,f,o,o,g,g): one 6-range sigmoid covers i,f,o
    and one 2-range tanh covers g. ACT/DVE ops are per-dir with fwd-first
    priority ordering (tg_f, sig_f, tg_b, tc_f, sig_b, tc_b) so the fwd
    recurrent matmuls of the next step unblock as early as possible.
  * All state transposed: h [128, 2*KH*NS] bf16, c [...] f32, in place.
    Final step writes f32 outputs, DMA'd out per dir.

kernel(**inputs) takes the FULL unsharded inputs and returns [2048, 512] f32.
"""
import numpy as np

import concourse.tile as tile
from concourse import bacc, mybir
from concourse.bass_utils import run_bass_kernel_spmd
import bass_rust

F32 = mybir.dt.float32
BF16 = mybir.dt.bfloat16
FP8 = mybir.dt.float8e4
DR = mybir.MatmulPerfMode.DoubleRow
AF = mybir.ActivationFunctionType
OP = mybir.AluOpType

USE_DR = True              # fp8 DoubleRow recurrent projection for V >= DRTHR
DRTHR = 160

N, T, D, H = 2048, 64, 512, 256
NCORES = 8
NS = N // NCORES           # 256 sequences per core
FH = 4 * H                 # 1024 gate rows
KD = D // 128              # 4 x-projection K chunks
KX = KD + 1                # + bias/mask chunk
KH = H // 128              # 2 h-projection K chunks
FORCE = -40.0              # gate penalty for pad steps
DIRS = ("f", "b")
# region r holds gate rows ROWMAP[r]:+128 : regions (i,i,f,f,o,o,g,g)
ROWMAP = [0, 128, 256, 384, 768, 896, 512, 640]
BLKW = 128                 # block column budget (2 PSUM banks per dir)
VTHR = 82                  # pack steps with V < VTHR (matmul issue floor)

_NC_CACHE = {}


def _inst(r):
    return getattr(r, "ins", r)


def _plan(t_steps, V):
    """Partition steps into packed blocks (V < VTHR) and per-step era."""
    blocks = []   # [s0, [v...], [intra off...], W]
    rsteps = []
    cur = None
    for s in range(t_steps):
        v = int(V[s])
        if v < VTHR:
            if cur is None or cur[3] + v > BLKW:
                cur = [s, [], [], 0]
                blocks.append(cur)
            cur[1].append(v)
            cur[2].append(cur[3])
            cur[3] += v
        else:
            cur = None
            rsteps.append(s)
    # single steps with V <= 128 fit 8 regions in 2 banks: run them on a
    # per-step parity ping-pong so next-step x matmuls never WAR-wait on
    # this step's ACT reads (the conflict lags a full step instead)
    keep = []
    psteps = []
    for b in blocks:
        if len(b[1]) >= 2:
            keep.append(b)
        else:
            psteps.append(b[0])
    rsteps += psteps
    rsteps.sort()
    return keep, [], rsteps


def _build(t_steps, V):
    import contextlib

    nc = bacc.Bacc("TRN2", target_bir_lowering=False, debug=False)
    V = [int(v) for v in V]
    OFF = np.concatenate([[0], np.cumsum(V)]).astype(int)
    TOT = int(OFF[t_steps])
    blocks, psteps, rsteps = _plan(t_steps, V)

    use_dr = USE_DR and any(V[s] >= DRTHR for s in range(t_steps))

    xp_d, wih_d, whh_d, whh8_d, out_d = {}, {}, {}, {}, {}
    for d in DIRS:
        xp_d[d] = nc.dram_tensor(
            f"xp{d}", [128, KX, TOT], BF16, kind="ExternalInput"
        ).ap()
        wih_d[d] = nc.dram_tensor(
            f"wih{d}", [128, KX, FH], BF16, kind="ExternalInput"
        ).ap()
        whh_d[d] = nc.dram_tensor(
            f"whh{d}", [128, KH, FH], BF16, kind="ExternalInput"
        ).ap()
        if use_dr:
            whh8_d[d] = nc.dram_tensor(
                f"whh8{d}", [128, KH, FH], FP8, kind="ExternalInput"
            ).ap()
        out_d[d] = nc.dram_tensor(
            f"hT{d}", [128, KH * NS], F32, kind="ExternalOutput"
        ).ap()

    with tile.TileContext(nc) as tc:
        with contextlib.ExitStack() as ctx:
            wpool = ctx.enter_context(tc.tile_pool(name="w", bufs=1))
            xpool = ctx.enter_context(tc.tile_pool(name="x", bufs=3))
            xbpool = ctx.enter_context(tc.tile_pool(name="xb", bufs=1))
            spool = ctx.enter_context(tc.tile_pool(name="state", bufs=1))
            opool = ctx.enter_context(tc.tile_pool(name="outs", bufs=1))
            apool = ctx.enter_context(tc.tile_pool(name="acts", bufs=2))
            pspool = ctx.enter_context(tc.tile_pool(name="ps", bufs=1, space="PSUM"))

            # warm-up weights first so the PE p-state ramp starts ASAP
            wrm = wpool.tile([128, 512], BF16, tag="warm", name="warm")
            nc.gpsimd.memset(wrm[:], 0.0)

            wih_t, whh_t, whh8_t = {}, {}, {}
            for d in DIRS:
                wih_t[d] = wpool.tile([128, KX, FH], BF16, tag=f"wih_{d}", name=f"wih_{d}")
                nc.gpsimd.dma_start(wih_t[d][:], wih_d[d][:])
                whh_t[d] = wpool.tile([128, KH, FH], BF16, tag=f"whh_{d}", name=f"whh_{d}")
                nc.gpsimd.dma_start(whh_t[d][:], whh_d[d][:])
            for d in DIRS:
                if use_dr:
                    whh8_t[d] = wpool.tile(
                        [128, KH, FH], FP8, tag=f"whh8_{d}", name=f"whh8_{d}"
                    )
                    nc.gpsimd.dma_start(whh8_t[d][:], whh8_d[d][:])

            # persistent transposed state, dir-major: [128, (d, kk, NS)]
            h_t = spool.tile([128, 2 * KH * NS], BF16, tag="h", name="h")
            nc.vector.memset(h_t[:], 0.0)
            c_t = spool.tile([128, 2 * KH * NS], F32, tag="c", name="c")
            nc.vector.memset(c_t[:], 0.0)
            h8_t = None
            if use_dr:
                h8_t = spool.tile([128, 2 * KH * NS], FP8, tag="h8", name="h8")
                nc.vector.memset(h8_t[:], 0.0)

            # single PSUM tile covering all 8 banks
            ps = pspool.tile([128, 8 * 512], F32, tag="ps", name="ps")

            # per-dir activation scratch (widths up to 256)
            tgA, sgA, tcA, t1A, t2A = {}, {}, {}, {}, {}
            for d in DIRS:
                tgA[d] = apool.tile([128, 2, 256], F32, tag=f"tg{d}", name=f"tg{d}")
                sgA[d] = apool.tile([128, 6, 256], F32, tag=f"sg{d}", name=f"sg{d}")
                tcA[d] = apool.tile([128, 2, 256], F32, tag=f"tc{d}", name=f"tc{d}")
                t1A[d] = apool.tile([128, 2, 256], F32, tag=f"t1{d}", name=f"t1{d}")
                t2A[d] = apool.tile([128, 2, 256], F32, tag=f"t2{d}", name=f"t2{d}")

            def hsl(d, kk, v):
                di = DIRS.index(d)
                base = di * KH * NS + kk * NS
                return h_t[:, base : base + v]

            def cviewd(d, v):
                di = DIRS.index(d)
                base = di * KH * NS
                return c_t[:, base : base + KH * NS].rearrange(
                    "p (k n) -> p k n", k=KH
                )[:, :, :v]

            def hviewd(d, v):
                di = DIRS.index(d)
                base = di * KH * NS
                return h_t[:, base : base + KH * NS].rearrange(
                    "p (k n) -> p k n", k=KH
                )[:, :, :v]

            def h8viewd(d, v):
                di = DIRS.index(d)
                base = di * KH * NS
                return h8_t[:, base : base + KH * NS].rearrange(
                    "p (k n) -> p k n", k=KH
                )[:, :, :v]

            rset = set(rsteps)

            def dr_step(s):
                return use_dr and s < t_steps and s in rset and V[s] >= DRTHR

            def emit_chain(s, v, psg, pss, last, split_sig=False):
                """Per-step activations+state update; psg[d]/pss[d] are the
                g / ifo PSUM views [p, r, v]. fwd-first priority ordering.
                split_sig: 3 per-bank sigmoid ops so the next step's x
                matmuls get their PSUM banks back progressively."""
                f, b = DIRS
                for d in DIRS:
                    nc.scalar.activation(tgA[d][:, :, :v], psg[d], AF.Tanh)
                    nc.scalar.activation(
                        sgA[d][:, :, :v], pss[d], AF.Sigmoid
                    )
                if last:
                    # kk-half chains so the output DMA overlaps the rest
                    for d in DIRS:
                        hfin = opool.tile(
                            [128, KH * NS], F32, tag=f"hfin{d}", name=f"hfin{d}"
                        )
                        qeng = [nc.sync, nc.scalar, nc.gpsimd, nc.sync]
                        for kk in range(KH):
                            k1 = kk + 1
                            nc.vector.tensor_tensor(
                                t1A[d][:, kk:k1, :v], sgA[d][:, kk:k1, :v],
                                tgA[d][:, kk:k1, :v], OP.mult,
                            )
                            nc.vector.tensor_tensor(
                                t2A[d][:, kk:k1, :v],
                                sgA[d][:, 2 + kk : 3 + kk, :v],
                                cviewd(d, v)[:, kk:k1, :], OP.mult,
                            )
                            nc.vector.tensor_tensor(
                                cviewd(d, v)[:, kk:k1, :],
                                t1A[d][:, kk:k1, :v], t2A[d][:, kk:k1, :v],
                                OP.add,
                            )
                            nc.scalar.activation(
                                tcA[d][:, kk:k1, :v],
                                cviewd(d, v)[:, kk:k1, :], AF.Tanh,
                            )
                            nc.vector.tensor_tensor(
                                hfin[:, kk * NS : kk * NS + v],
                                sgA[d][:, 4 + kk, :v],
                                tcA[d][:, kk, :v],
                                OP.mult,
                            )
                            for qi in range(2):
                                CH = NS // 2
                                off = kk * NS + qi * CH
                                qeng[2 * kk + qi].dma_start(
                                    out_d[d][:, off : off + CH],
                                    hfin[:, off : off + CH],
                                )
                    return
                for d in DIRS:
                    nc.vector.tensor_tensor(
                        t1A[d][:, :, :v], sgA[d][:, 0:2, :v], tgA[d][:, :, :v],
                        OP.mult,
                    )
                    nc.vector.tensor_tensor(
                        t2A[d][:, :, :v], sgA[d][:, 2:4, :v], cviewd(d, v),
                        OP.mult,
                    )
                    nc.vector.tensor_tensor(
                        cviewd(d, v), t1A[d][:, :, :v], t2A[d][:, :, :v], OP.add
                    )
                    nc.scalar.activation(tcA[d][:, :, :v], cviewd(d, v), AF.Tanh)
                    tgt = (
                        h8viewd(d, v) if dr_step(s + 1) else hviewd(d, v)
                    )
                    nc.vector.tensor_tensor(
                        tgt, sgA[d][:, 4:6, :v], tcA[d][:, :, :v],
                        OP.mult,
                    )

            # ---- PE warm-up (p-state ramp) during initial DMA window ----
            NWARM = 18
            for i in range(NWARM):
                nc.tensor.matmul(
                    ps[:, 3584:4096], wrm[:, 0:128], wrm[:],
                    start=(i == 0), stop=(i == NWARM - 1),
                )

            # ================= block era (V < VTHR) =================
            # psum: dir d, parity q: base = d*2048 + q*1024, region r at r*128
            # all blocks are contiguous packed columns: ONE big DMA per dir
            # has ~7x larger segments than per-block slices (descriptor-bound)
            xall = {}
            BTOT = 0
            if blocks:
                bs_end, bvs, bofs, bW = blocks[-1]
                BTOT = int(OFF[bs_end] + bW)
                for d in DIRS:
                    xall[d] = xbpool.tile(
                        [128, KX, BTOT], BF16, tag=f"x_{d}", name=f"x_{d}"
                    )
                    nc.sync.dma_start(xall[d][:], xp_d[d][:, :, 0:BTOT])
            # second contiguous prefetch covering the early per-step era
            ESZ = min(2432, TOT - BTOT)
            xearly = {}
            if ESZ > 0:
                for d in DIRS:
                    xearly[d] = xbpool.tile(
                        [128, KX, ESZ], BF16, tag=f"xe_{d}", name=f"xe_{d}"
                    )
                    nc.sync.dma_start(
                        xearly[d][:], xp_d[d][:, :, BTOT : BTOT + ESZ]
                    )

            def dma_block(bi):
                pass

            dma_gate = [None]

            def xproj_block(bi):
                s0, _, _, W = blocks[bi]
                q = bi % 2
                for d in DIRS:
                    di = DIRS.index(d)
                    base = di * 2048 + q * 1024
                    bank_start = {}
                    for r in range(8):
                        o_ap = ps[:, base + r * 128 : base + r * 128 + W]
                        msl = slice(ROWMAP[r], ROWMAP[r] + 128)
                        for k in range(KX):
                            mm = nc.tensor.matmul(
                                o_ap, wih_t[d][:, k, msl],
                                xall[d][:, k, OFF[s0] : OFF[s0] + W],
                                start=(r % 4 == 0 and k == 0), stop=False,
                                skip_group_check=True,
                            )
                            if r % 4 == 0 and k == 0:
                                bank_start[r // 4] = _inst(mm)
                                if bi == min(2, len(blocks) - 1):
                                    dma_gate[0] = _inst(mm)
                            elif k == 0:
                                bass_rust.add_dep_helper(
                                    _inst(mm), bank_start[r // 4], sync=False,
                                    reason="psum bank group order",
                                )

            def psB(d, q, lo, hi, off, v):
                di = DIRS.index(d)
                base = di * 2048 + q * 1024
                return (
                    ps[:, base + lo * 128 : base + hi * 128]
                    .rearrange("p (r c) -> p r c", r=hi - lo)[:, :, off : off + v]
                )

            # merged-dir views/scratch for tiny block steps: fixed per-op
            # costs dominate there, so one op covers both directions
            def psBM(q, lo, hi, off, v):
                return (
                    ps[:]
                    .rearrange("p (d u) -> p d u", d=2)[
                        :, :, q * 1024 + lo * 128 : q * 1024 + hi * 128
                    ]
                    .rearrange("p d (r c) -> p d r c", r=hi - lo)[
                        :, :, :, off : off + v
                    ]
                )

            def cviewm(v):
                return c_t[:].rearrange("p (d k n) -> p d k n", d=2, k=KH)[
                    :, :, :, :v
                ]

            def hviewm(v):
                return h_t[:].rearrange("p (d k n) -> p d k n", d=2, k=KH)[
                    :, :, :, :v
                ]

            tgM = apool.tile([128, 2, 2, BLKW], F32, tag="tgM", name="tgM")
            sgM = apool.tile([128, 2, 6, BLKW], F32, tag="sgM", name="sgM")
            tcM = apool.tile([128, 2, 2, BLKW], F32, tag="tcM", name="tcM")
            t1M = apool.tile([128, 2, 2, BLKW], F32, tag="t1M", name="t1M")
            t2M = apool.tile([128, 2, 2, BLKW], F32, tag="t2M", name="t2M")

            def emit_chain_merged(q, off, v):
                nc.scalar.activation(
                    tgM[:, :, :, :v], psBM(q, 6, 8, off, v), AF.Tanh
                )
                nc.scalar.activation(
                    sgM[:, :, :, :v], psBM(q, 0, 6, off, v), AF.Sigmoid
                )
                nc.vector.tensor_tensor(
                    t1M[:, :, :, :v], sgM[:, :, 0:2, :v], tgM[:, :, :, :v],
                    OP.mult,
                )
                nc.vector.tensor_tensor(
                    t2M[:, :, :, :v], sgM[:, :, 2:4, :v], cviewm(v), OP.mult
                )
                nc.vector.tensor_tensor(
                    cviewm(v), t1M[:, :, :, :v], t2M[:, :, :, :v], OP.add
                )
                nc.scalar.activation(tcM[:, :, :, :v], cviewm(v), AF.Tanh)
                nc.vector.tensor_tensor(
                    hviewm(v), sgM[:, :, 4:6, :v], tcM[:, :, :, :v], OP.mult
                )

            if blocks:
                xproj_block(0)
                if len(blocks) > 1:
                    xproj_block(1)
            for bi, (s0, vs, offs, W) in enumerate(blocks):
                q = bi % 2
                for si, (v, off) in enumerate(zip(vs, offs)):
                    s = s0 + si
                    last = s == t_steps - 1
                    for d in DIRS:
                        di = DIRS.index(d)
                        base = di * 2048 + q * 1024
                        for r in range(8):
                            o_ap = ps[
                                :, base + r * 128 + off : base + r * 128 + off + v
                            ]
                            msl = slice(ROWMAP[r], ROWMAP[r] + 128)
                            for kk in range(KH):
                                nc.tensor.matmul(
                                    o_ap,
                                    whh_t[d][:, kk, msl],
                                    hsl(d, kk, v),
                                    start=False,
                                    stop=(kk == KH - 1),
                                    skip_group_check=True,
                                )
                    emit_chain(
                        s, v,
                        {d: psB(d, q, 6, 8, off, v) for d in DIRS},
                        {d: psB(d, q, 0, 6, off, v) for d in DIRS},
                        last,
                    )
                if bi + 2 < len(blocks):
                    xproj_block(bi + 2)


            # ========== parity-step era (V <= 128, single steps) ==========
            # block layout per step, parity alternates; xproj(s+1) is
            # emitted BEFORE hproj(s) so its PSUM WAR lags a full step
            def xproj_P(s, q, xt):
                v = V[s]
                for d in DIRS:
                    di = DIRS.index(d)
                    base = di * 2048 + q * 1024
                    bank_start = {}
                    for r in range(8):
                        o_ap = ps[:, base + r * 128 : base + r * 128 + v]
                        msl = slice(ROWMAP[r], ROWMAP[r] + 128)
                        for k in range(KX):
                            mm = nc.tensor.matmul(
                                o_ap, wih_t[d][:, k, msl], xt[d][:, k, :v],
                                start=(r % 4 == 0 and k == 0), stop=False,
                                skip_group_check=True,
                            )
                            if r % 4 == 0 and k == 0:
                                bank_start[r // 4] = _inst(mm)
                            elif k == 0:
                                bass_rust.add_dep_helper(
                                    _inst(mm), bank_start[r // 4], sync=False,
                                    reason="psum bank group order",
                                )

            def hproj_P(s, q):
                v = V[s]
                for d in DIRS:
                    di = DIRS.index(d)
                    base = di * 2048 + q * 1024
                    for r in range(8):
                        o_ap = ps[:, base + r * 128 : base + r * 128 + v]
                        msl = slice(ROWMAP[r], ROWMAP[r] + 128)
                        for kk in range(KH):
                            nc.tensor.matmul(
                                o_ap, whh_t[d][:, kk, msl], hsl(d, kk, v),
                                start=False, stop=(kk == KH - 1),
                                skip_group_check=True,
                            )

            px_tiles = {}

            def dma_pstep(s):
                px_tiles[s] = {}
                for d in DIRS:
                    t = xpool.tile(
                        [128, KX, BLKW], BF16, tag=f"xP_{d}", name=f"xP_{d}"
                    )
                    nc.sync.dma_start(
                        t[:, :, : V[s]], xp_d[d][:, :, OFF[s] : OFF[s] + V[s]]
                    )
                    px_tiles[s][d] = t

            for pi, s in enumerate(psteps):
                v = V[s]
                q = pi % 2
                if pi == 0:
                    dma_pstep(s)
                    xproj_P(s, q, px_tiles[s])
                if pi + 1 < len(psteps):
                    nx = psteps[pi + 1]
                    dma_pstep(nx)
                    xproj_P(nx, (pi + 1) % 2, px_tiles[nx])
                hproj_P(s, q)
                emit_chain(
                    s, v,
                    {d: psB(d, q, 6, 8, 0, v) for d in DIRS},
                    {d: psB(d, q, 0, 6, 0, v) for d in DIRS},
                    s == t_steps - 1,
                )
                px_tiles.pop(s, None)

            # ================= per-step era (V > 128) =================
            # psum: dir d base = d*2048, region r at r*256; banks = region pairs
            def psR(d, lo, hi, v):
                di = DIRS.index(d)
                return (
                    ps[:, di * 2048 + lo * 256 : di * 2048 + hi * 256]
                    .rearrange("p (r c) -> p r c", r=hi - lo)[:, :, :v]
                )

            def xproj_R(d, s, v, xt):
                di = DIRS.index(d)
                base = di * 2048
                starts = {}
                for bank in (3, 0, 1, 2):
                    for rr in range(2):
                        r = bank * 2 + rr
                        o_ap = ps[:, base + r * 256 : base + r * 256 + v]
                        msl = slice(ROWMAP[r], ROWMAP[r] + 128)
                        for k in range(KX):
                            mm = nc.tensor.matmul(
                                o_ap, wih_t[d][:, k, msl], xt[:, k, :v],
                                start=(rr == 0 and k == 0), stop=False,
                                skip_group_check=True,
                            )
                            if rr == 0 and k == 0:
                                starts[bank] = _inst(mm)
                            elif k == 0:
                                bass_rust.add_dep_helper(
                                    _inst(mm), starts[bank], sync=False,
                                    reason="psum bank group order",
                                )

            def hproj_R(d, v, dr):
                di = DIRS.index(d)
                base = di * 2048
                for bank in (3, 0, 1, 2):
                    for rr in range(2):
                        r = bank * 2 + rr
                        o_ap = ps[:, base + r * 256 : base + r * 256 + v]
                        msl = slice(ROWMAP[r], ROWMAP[r] + 128)
                        if dr:
                            nc.tensor.matmul(
                                o_ap, whh8_t[d][:, :, msl], h8viewd(d, v),
                                start=False, stop=(rr == 1),
                                perf_mode=DR,
                                skip_group_check=True,
                            )
                        else:
                            for kk in range(KH):
                                nc.tensor.matmul(
                                    o_ap, whh_t[d][:, kk, msl], hsl(d, kk, v),
                                    start=False,
                                    stop=(rr == 1 and kk == KH - 1),
                                    skip_group_check=True,
                                )

            for s in rsteps:
                v = V[s]
                last = s == t_steps - 1
                if OFF[s] >= BTOT and OFF[s] + v <= BTOT + ESZ:
                    xts = {d: xearly[d][:, :, OFF[s] - BTOT :] for d in DIRS}
                else:
                    xts = {}
                    for d in DIRS:
                        xts[d] = xpool.tile(
                            [128, KX, 256], BF16, tag=f"xR_{d}", name=f"xR_{d}"
                        )
                        dm = nc.sync.dma_start(
                            xts[d][:, :, :v], xp_d[d][:, :, OFF[s] : OFF[s] + v]
                        )
                        if dma_gate[0] is not None:
                            # keep bulk prefetch off the DMA hardware until
                            # the startup transfers have drained
                            bass_rust.add_dep_helper(
                                _inst(dm), dma_gate[0], sync=True,
                                reason="throttle steady-state prefetch",
                            )
                            dma_gate[0] = None
                for d in DIRS:
                    xproj_R(d, s, v, xts[d])
                for d in DIRS:
                    hproj_R(d, v, dr_step(s))
                emit_chain(
                    s, v,
                    {d: psR(d, 6, 8, v) for d in DIRS},
                    {d: psR(d, 0, 6, v) for d in DIRS},
                    last,
                )

    nc.compile()
    return nc


def _get_nc(t_steps, V):
    key = (t_steps, tuple(int(v) for v in V), USE_DR, DRTHR)
    if key not in _NC_CACHE:
        _NC_CACHE[key] = _build(t_steps, V)
    return _NC_CACHE[key]


def _prep_weights(W_ih, W_hh, b):
    """lhsT layouts for one direction, bias/mask folded as x chunk KD."""
    import ml_dtypes

    wdt = ml_dtypes.bfloat16
    wih = np.zeros((128, KX, FH), np.float32)
    wih[:, :KD, :] = W_ih.T.reshape(128, KD, FH)   # (p, k) <-> dd = KD*p + k
    coef = np.zeros(FH, np.float32)
    coef[: 2 * H] = FORCE       # i, f gates
    coef[3 * H :] = FORCE       # o gate
    wih[0, KD, :] = b.astype(np.float32)
    wih[1, KD, :] = coef
    wih = np.ascontiguousarray(wih.astype(wdt))
    whhf = W_hh.T.reshape(KH, 128, FH).transpose(1, 0, 2)
    whh = np.ascontiguousarray(whhf.astype(wdt))   # (p, kk) <-> hrow = 128*kk + p
    whh8 = np.ascontiguousarray(whhf.astype(ml_dtypes.float8_e4m3fn))
    return wih, whh, whh8


def _prep_core(seqs_c, lens_c, t_steps, V, OFF):
    """Packed per-core x arrays. seqs_c [NS, T, D], lens_c [NS] (sorted desc)."""
    import ml_dtypes

    bf16 = ml_dtypes.bfloat16
    TOTC = int(OFF[t_steps])
    shift = t_steps - lens_c                                   # pad steps/seq
    rows = np.concatenate([np.arange(V[s]) for s in range(t_steps)])
    steps = np.repeat(np.arange(t_steps), V[:t_steps])
    pad = steps < shift[rows]                                  # not yet started

    out = {}
    for dname, rev in (("f", False), ("b", True)):
        if rev:
            tok = t_steps - 1 - steps
        else:
            tok = steps - shift[rows]
        tokc = np.clip(tok, 0, T - 1)
        gat = seqs_c[rows, tokc]                               # [TOTC, D]
        if not rev:
            gat = np.where(pad[:, None], np.float32(0.0), gat)
        arr = np.zeros((128, KX, TOTC), np.float32)
        arr[:, :KD, :] = gat.T.reshape(128, KD, TOTC)          # dd = KD*p + k
        arr[0, KD, :] = 1.0
        arr[1, KD, :] = pad.astype(np.float32)
        out[dname] = np.ascontiguousarray(arr.astype(bf16))
    return out


def _unfold(hT):
    """[128, KH*NS] device tile -> [NS, H] h matrix."""
    h_rows = np.concatenate([hT[:, i * NS : (i + 1) * NS] for i in range(KH)], axis=0)
    return h_rows.T  # [NS, H]


def _run(inputs, trace=False, t_cap=None, **spmd_kwargs):
    all_embs = np.asarray(inputs["all_embs"], dtype=np.float32)
    lengths = np.asarray(inputs["lengths"]).astype(np.int64)
    starts = np.asarray(inputs["starts"]).astype(np.int64)

    if np.array_equal(starts, np.arange(N, dtype=np.int64) * T):
        seqs = all_embs.reshape(N, T, D)
    else:
        seqs = all_embs[starts[:, None] + np.arange(T)[None, :]]

    # global sort by length desc, deal round-robin to cores
    order = np.argsort(-lengths, kind="stable")
    t_steps = int(lengths.max())
    if t_cap is not None:
        t_steps = min(t_steps, t_cap)
    core_idx = [order[c::NCORES] for c in range(NCORES)]  # [NCORES][NS]

    # baked active widths: V_s = max over cores of #{len >= t_steps - s}
    Ls = np.stack([np.minimum(lengths[ci], t_steps) for ci in core_idx])  # [NC, NS]
    thr = t_steps - np.arange(t_steps)  # [t]
    V = (Ls[:, None, :] >= thr[None, :, None]).sum(-1).max(0)  # [t]
    V = np.maximum(V, 1).astype(int)
    OFF = np.concatenate([[0], np.cumsum(V)]).astype(int)

    w = {}
    for d, (wi, wh, bb) in {
        "f": (inputs["W_ih_f"], inputs["W_hh_f"], inputs["b_f"]),
        "b": (inputs["W_ih_b"], inputs["W_hh_b"], inputs["b_b"]),
    }.items():
        w[d] = _prep_weights(
            np.asarray(wi, np.float32), np.asarray(wh, np.float32),
            np.asarray(bb, np.float32),
        )

    in_maps = []
    for ci in range(NCORES):
        idx = core_idx[ci]
        m = _prep_core(seqs[idx], np.minimum(lengths[idx], t_steps), t_steps, V, OFF)
        im = {
            "xpf": m["f"], "xpb": m["b"],
            "wihf": w["f"][0], "whhf": w["f"][1],
            "wihb": w["b"][0], "whhb": w["b"][1],
        }
        if USE_DR and any(int(v) >= DRTHR for v in V):
            im["whh8f"] = w["f"][2]
            im["whh8b"] = w["b"][2]
        in_maps.append(im)

    nc = _get_nc(t_steps, V)
    res = None
    for attempt in range(3):
        try:
            res = run_bass_kernel_spmd(
                nc, in_maps, core_ids=list(range(NCORES)), trace=trace,
                **spmd_kwargs
            )
            break
        except Exception:
            # rare transient NRT_EXEC_UNIT_UNRECOVERABLE right after a
            # fresh NEFF load; a plain re-execute has always recovered
            if attempt == 2:
                raise
            import time as _time

            _time.sleep(2.0)

    out = np.empty((N, 2 * H), np.float32)
    for ci in range(NCORES):
        out[core_idx[ci], :H] = _unfold(res.results[ci]["hTf"])
        out[core_idx[ci], H:] = _unfold(res.results[ci]["hTb"])
    return out, res


def kernel(**inputs) -> np.ndarray:
    out, _ = _run(inputs)
    return out
